# revision 7
# baseline (speedup 1.0000x reference)
"""Trainium2 Bass kernel for nn_GaussianDerivativeESPLayer.

Strategy (per the data-parallel hint, extended since B=4 < 8 cores):
shard (batch b, H-half) across the 8 cores. Each core computes output
rows [H0, H0+93) of one batch element from input rows [g0, g0+105).

Per-core pipeline (all layouts [partition, free]):
  1. yconv: data-stationary matmuls X^T @ yband -> Z [w, (fy, c, h')]
     (the 6 Gaussian-derivative y-factors for both sigmas at once).
  2. xconv: band-stationary matmuls xband^T @ Z -> L [w', (c, h')]
     (12 separable derivative maps, 6 per sigma).
  3. Gram: per-channel products (ACT squares for diagonal entries, DVE
     fused mult for crosses) + fold-tree accumulation over channels
     -> 20 S maps [w', h'] (10 Gram entries per sigma).
  4. Integration: valid 7x7 separable conv as two matmul stages
     (x-int via S-stationary, y-int via band-stationary) -> [h'', w''].
  5. ESP: Newton-identity elementary symmetric polynomials in closed
     form per pixel, roots via exp/ln -> 12 output channels.
"""

import math
import os

import numpy as np

B, H, W, C = 4, 192, 192, 64
NH = 99  # h' rows computed per core (pre y-integration)
NOUT = 93  # h'' output rows per core
HL = 105  # input rows per core
NWO = 186  # output cols
CBLK = 8  # channels per block
NBLK = C // CBLK
CSUB = 4  # channels per xconv matmul chunk
RADII = [3, 6]  # sigma radii
RI = 3  # integrator radius
EPS = float(np.finfo(np.float64).eps)

# conv matmul dtype: "fp32" (exact, 4 cyc/col) or "fp32r" (TF32-ish, 1 cyc/col)
CONV_MODE = os.environ.get("GK_CONV_DT", "fp32")

_CACHE = {}
RUN_KWARGS = {}  # test harness can set dict(trace=True) before calling kernel()
LAST = None  # BassKernelResults of the most recent kernel() call


# ---------------------------------------------------------------- host math
def _extract_filters(kernels, sigma_idx):
    """kernels [6, kh, kw, C, 1] channel-tiled separable. Returns
    (gys, gxs): 1D filters with sigma^order folded in, such that
    kernel(j,k) == outer(gys[j-k], gxs[k])."""
    K = np.asarray(kernels, dtype=np.float64)[:, :, :, 0, 0]
    i0 = K.shape[1] // 2
    s0 = math.sqrt(abs(K[0][i0, i0]))
    g0y = K[0][:, i0] / s0
    g0x = K[0][i0, :] / s0
    g1x = K[2][i0, :] / g0y[i0]  # (j=1,k=1) = s*g1x (x) * g0y (y)
    g1y = K[1][:, i0] / g0x[i0]  # (j=1,k=0) = s*g0x (x) * g1y (y)
    g2x = K[5][i0, :] / g0y[i0]
    g2y = K[3][:, i0] / g0x[i0]
    return [g0y, g1y, g2y], [g0x, g1x, g2x]


def _extract_integrator(dg_int):
    K = np.asarray(dg_int, dtype=np.float64)[:, :, 0, 0]
    i0 = K.shape[0] // 2
    s0 = math.sqrt(abs(K[i0, i0]))
    return K[:, i0] / s0, K[i0, :] / s0  # giy, gix


def _band(k_count, m_count, g, delta):
    """band[k, m] = g[k - m + delta + r] when |k - m + delta| <= r."""
    r = len(g) // 2
    k = np.arange(k_count)[:, None]
    m = np.arange(m_count)[None, :]
    d = k - m + delta
    ok = np.abs(d) <= r
    out = np.zeros((k_count, m_count), dtype=np.float64)
    out[ok] = np.asarray(g)[(d + r)[ok]]
    return out


def _build_host_tensors(kernels0, kernels1, dg_int):
    gys0, gxs0 = _extract_filters(kernels0, 0)
    gys1, gxs1 = _extract_filters(kernels1, 1)
    giy, gix = _extract_integrator(dg_int)
    gys = [gys0, gys1]
    gxs = [gxs0, gxs1]

    # yband per half: [HL, 594] cols = (sigma, fy, h'-local)
    ybands = []
    for half in range(2):
        H0 = half * NOUT
        g0 = 0 if half == 0 else 87
        cols = []
        for s in range(2):
            for fy in range(3):
                cols.append(_band(HL, NH, gys[s][fy], g0 - H0))
        ybands.append(np.concatenate(cols, axis=1).astype(np.float32))

    # xband: [102, 1152] cols = (sigma, half, fx, 96)  -- core-independent
    xcols = []
    for s in range(2):
        for hf in range(2):
            delta = 0 if hf == 0 else (90 - 96)
            for fx in range(3):
                xcols.append(_band(102, 96, gxs[s][fx], delta))
    xband = np.concatenate(xcols, axis=1).astype(np.float32)

    # x-integration bands [96, 372]: valid conv, out w'' n: sum_k S[w'=k+96*hf]*gix[w'-n]
    # band[k, n] = gix[k + 96*hf - n], need 0 <= k + 96*hf - n <= 6
    ix = []
    for hf in range(2):
        k = np.arange(96)[:, None] + 96 * hf
        n = np.arange(NWO)[None, :]
        d = k - n
        ok = (d >= 0) & (d <= 2 * RI)
        b = np.zeros((96, NWO))
        b[ok] = gix[d[ok]]
        ix.append(b)
    intx = np.concatenate(ix, axis=1).astype(np.float32)

    # y-integration band [NH, NOUT]: inty[k, m] = giy[k - m], 0 <= k-m <= 6
    k = np.arange(NH)[:, None]
    m = np.arange(NOUT)[None, :]
    d = k - m
    ok = (d >= 0) & (d <= 2 * RI)
    inty = np.zeros((NH, NOUT))
    inty[ok] = giy[d[ok]]
    inty = inty.astype(np.float32)

    return ybands, xband, intx, inty


# pairs per sigma: list of (m1, m2, weight, is_diag) with m = l-index (x-order k)
# maps: l-index k of level j -> (ky=j-k, kx=k); map id = index into L[s][.]
# L map order: 6 maps: (j,k): (0,0),(1,0),(1,1),(2,0),(2,1),(2,2)
_LMAP = {(0, 0): 0, (1, 0): 1, (1, 1): 2, (2, 0): 3, (2, 1): 4, (2, 2): 5}
# map id -> (fy, fx) = (j-k, k)
_MAP_FYFX = {0: (0, 0), 1: (1, 0), 2: (0, 1), 3: (2, 0), 4: (1, 1), 5: (0, 2)}


def _pair_list():
    pairs = []  # (sid, m1, m2, w, diag) ; S index = position
    for j in range(3):
        for k1 in range(j + 1):
            for k2 in range(k1, j + 1):
                w = math.sqrt(math.comb(j, k1) * math.comb(j, k2))
                pairs.append((_LMAP[(j, k1)], _LMAP[(j, k2)], w, k1 == k2))
    return pairs  # 10 per sigma


_PAIRS = _pair_list()
# S indices per sigma: j0: [0]; j1: A=1 B=2 D=3 ; j2: A=4 B=5 C=6 D=7 E=8 F=9


# ---------------------------------------------------------------- bass build
def _build_module():
    import concourse.bacc as bacc
    import concourse.mybir as mybir
    import concourse.tile as tile

    f32 = mybir.dt.float32
    cdt = mybir.dt.float32r if CONV_MODE == "fp32r" else f32
    AF = mybir.ActivationFunctionType
    OP = mybir.AluOpType

    nc = bacc.Bacc("TRN2", target_bir_lowering=False, debug=False, num_devices=8)
    x_d = nc.dram_tensor("x", [HL, C * W], cdt, kind="ExternalInput").ap()
    yb_d = nc.dram_tensor("yband", [HL, 594], cdt, kind="ExternalInput").ap()
    xb_d = nc.dram_tensor("xband", [102, 1152], cdt, kind="ExternalInput").ap()
    ix_d = nc.dram_tensor("intx", [96, 2 * NWO], f32, kind="ExternalInput").ap()
    iy_d = nc.dram_tensor("inty", [NH, NOUT], f32, kind="ExternalInput").ap()
    out_d = nc.dram_tensor("out", [NOUT, NWO * 12], f32, kind="ExternalOutput").ap()

    with tile.TileContext(nc) as tc:
        _emit(tc, nc, x_d, yb_d, xb_d, ix_d, iy_d, out_d, f32, cdt, AF, OP)
    nc.compile()
    return nc


def _emit(tc, nc, x_d, yb_d, xb_d, ix_d, iy_d, out_d, f32, cdt, AF, OP):
    import concourse.bass as bass  # noqa

    ES = None  # placeholder

    cpool = tc.alloc_tile_pool(name="consts", bufs=1)
    yb = cpool.tile([HL, 594], cdt, name="yb")
    nc.sync.dma_start(yb[:], yb_d[:])
    xb = cpool.tile([102, 1152], cdt, name="xb")
    nc.sync.dma_start(xb[:], xb_d[:])
    ixb = cpool.tile([96, 2 * NWO], f32, name="ixb")
    nc.sync.dma_start(ixb[:], ix_d[:])
    iyb = cpool.tile([NH, NOUT], f32, name="iyb")
    nc.sync.dma_start(iyb[:], iy_d[:])
    c_eps = cpool.tile([128, 1], f32, name="c_eps")
    nc.vector.memset(c_eps[:], EPS)
    c_ln10 = cpool.tile([128, 1], f32, name="c_ln10")
    nc.vector.memset(c_ln10[:], math.log(10.0))
    c_ln100 = cpool.tile([128, 1], f32, name="c_ln100")
    nc.vector.memset(c_ln100[:], math.log(100.0))
    b_eps = c_eps[:NOUT]
    b_ln10 = c_ln10[:NOUT]
    b_ln100 = c_ln100[:NOUT]

    spool = tc.alloc_tile_pool(name="smaps", bufs=1)
    S = {}
    for s in range(2):
        for p in range(10):
            for hf in range(2):
                S[(s, p, hf)] = spool.tile(
                    [96, NH], f32, name=f"S_{s}_{p}_{hf}", tag=f"S_{s}_{p}_{hf}"
                )

    xin = tc.alloc_tile_pool(name="xin", bufs=2)
    zsb = tc.alloc_tile_pool(name="zsb", bufs=1)
    lsb = tc.alloc_tile_pool(name="lsb", bufs=1)
    ppool = tc.alloc_tile_pool(name="ppool", bufs=4)
    fpool = tc.alloc_tile_pool(name="fpool", bufs=6)
    zps = tc.alloc_tile_pool(name="zps", bufs=3, space="PSUM")
    xps = tc.alloc_tile_pool(name="xps", bufs=4, space="PSUM")

    for blk in range(NBLK):
        xt = xin.tile([HL, CBLK * W], cdt, name="xt", tag="xt")
        nc.sync.dma_start(xt[:], x_d[:, blk * CBLK * W : (blk + 1) * CBLK * W])

        # ---- yconv: Z[s][hf] layout [102, 3, CBLK, NH]
        Z = {}
        for s in range(2):
            for hf in range(2):
                Z[(s, hf)] = zsb.tile(
                    [102, 3, CBLK, NH], cdt, name=f"z{s}{hf}", tag=f"z{s}{hf}"
                )
        for ci in range(CBLK):
            for hf in range(2):
                w0 = 0 if hf == 0 else 90
                lhs = xt[:, ci * W + w0 : ci * W + w0 + 102]
                for s in range(2):
                    zp = zps.tile([102, 297], f32, name="zp", tag="zp")
                    nc.tensor.matmul(
                        zp[:], lhs, yb[:, s * 297 : (s + 1) * 297], start=True, stop=True
                    )
                    src = zp[:].rearrange("p (f h) -> p f h", f=3)
                    nc.scalar.copy(Z[(s, hf)][:, :, ci, :], src)

        # ---- xconv: L[s][m] layout [96 w'-half, CBLK, NH] per half
        L = {}
        for s in range(2):
            for m in range(6):
                for hf in range(2):
                    L[(s, m, hf)] = lsb.tile(
                        [96, CBLK, NH], f32, name=f"l{s}{m}{hf}", tag=f"l{s}{m}{hf}"
                    )
        for s in range(2):
            for hf in range(2):
                for fx in range(3):
                    xb_col = (s * 2 + hf) * 3 + fx
                    lhsT = xb[:, xb_col * 96 : (xb_col + 1) * 96]
                    for fy in range(3 - fx):
                        m = _LMAP[(fy + fx, fx)]
                        for cs in range(CBLK // CSUB):
                            rhs = Z[(s, hf)][:, fy, cs * CSUB : (cs + 1) * CSUB, :]
                            xp = xps.tile([96, CSUB * NH], f32, name="xp", tag="xp")
                            nc.tensor.matmul(xp[:], lhsT, rhs, start=True, stop=True)
                            dst = L[(s, m, hf)][:, cs * CSUB : (cs + 1) * CSUB, :]
                            nc.scalar.copy(
                                dst, xp[:].rearrange("p (c h) -> p c h", c=CSUB)
                            )

        # ---- Gram products + fold over channels
        for s in range(2):
            for pi, (m1, m2, wgt, diag) in enumerate(_PAIRS):
                for hf in range(2):
                    P = ppool.tile([96, CBLK * NH], f32, name="P", tag="P")
                    a1 = L[(s, m1, hf)][:].rearrange("p c h -> p (c h)")
                    if diag:
                        nc.scalar.activation(
                            P[:], a1, AF.Square, bias=0.0, scale=math.sqrt(wgt)
                        )
                    else:
                        a2 = L[(s, m2, hf)][:].rearrange("p c h -> p (c h)")
                        nc.vector.scalar_tensor_tensor(
                            P[:], a1, wgt, a2, OP.mult, OP.mult
                        )
                    f1 = fpool.tile([96, CBLK * NH // 2], f32, name="f1", tag="f1")
                    nc.vector.tensor_add(
                        f1[:], P[:, : CBLK * NH // 2], P[:, CBLK * NH // 2 :]
                    )
                    f2 = fpool.tile([96, CBLK * NH // 4], f32, name="f2", tag="f2")
                    nc.vector.tensor_add(
                        f2[:], f1[:, : CBLK * NH // 4], f1[:, CBLK * NH // 4 :]
                    )
                    st = S[(s, pi, hf)]
                    if blk == 0:
                        nc.vector.tensor_add(st[:], f2[:, :NH], f2[:, NH:])
                    else:
                        f3 = fpool.tile([96, NH], f32, name="f3", tag="f3")
                        nc.vector.tensor_add(f3[:], f2[:, :NH], f2[:, NH:])
                        nc.vector.tensor_add(st[:], st[:], f3[:])

    for pool in (xps, zps, fpool, ppool, lsb, zsb, xin):
        pool.release()

    # ---- integration -> SM maps [NOUT, NWO]
    impool = tc.alloc_tile_pool(name="impool", bufs=1)
    SM = {}
    for s in range(2):
        for p in range(10):
            SM[(s, p)] = impool.tile(
                [NOUT, NWO], f32, name=f"SM_{s}_{p}", tag=f"SM_{s}_{p}"
            )
    sxp = tc.alloc_tile_pool(name="sxp", bufs=3)
    ips = tc.alloc_tile_pool(name="ips", bufs=2, space="PSUM")
    yps = tc.alloc_tile_pool(name="yps", bufs=2, space="PSUM")
    for s in range(2):
        for p in range(10):
            ip = ips.tile([NH, NWO], f32, name="ip", tag="ip")
            nc.tensor.matmul(
                ip[:], S[(s, p, 0)][:], ixb[:, :NWO], start=True, stop=False
            )
            nc.tensor.matmul(
                ip[:], S[(s, p, 1)][:], ixb[:, NWO:], start=False, stop=True
            )
            sx = sxp.tile([NH, NWO], f32, name="sx", tag="sx")
            nc.scalar.copy(sx[:], ip[:])
            yp = yps.tile([NOUT, NWO], f32, name="yp", tag="yp")
            nc.tensor.matmul(yp[:], iyb[:], sx[:], start=True, stop=True)
            nc.scalar.copy(SM[(s, p)][:], yp[:])
    for pool in (yps, ips, sxp):
        pool.release()

    # ---- ESP + output
    opool = tc.alloc_tile_pool(name="opool", bufs=1)
    OUTT = opool.tile([NOUT, NWO * 12], f32, name="OUTT")
    outv = OUTT[:].rearrange("p (w c) -> p w c", c=12)
    epool = tc.alloc_tile_pool(name="epool", bufs=2)

    def et(name):
        return epool.tile([NOUT, NWO], f32, name=name, tag=name)

    for s in range(2):
        ch0 = s * 6
        # ---- j = 0
        m0 = SM[(s, 0)]
        t0 = et("t0j0")
        nc.scalar.activation(t0[:], m0[:], AF.Abs)
        nc.vector.tensor_scalar(outv[:, :, ch0 + 0], t0[:], EPS, None, OP.add, OP.bypass)
        # ---- j = 1 : A=1 B=2 D=3
        A, Bm, D = SM[(s, 1)], SM[(s, 2)], SM[(s, 3)]
        p1 = et("p1j1")
        nc.vector.tensor_add(p1[:], A[:], D[:])
        t = et("tj1")
        nc.scalar.activation(t[:], p1[:], AF.Abs)
        nc.vector.tensor_scalar(outv[:, :, ch0 + 1], t[:], EPS, 10.0, OP.add, OP.mult)
        q = et("qj1")
        nc.vector.scalar_tensor_tensor(q[:], p1[:], 1.0, p1[:], OP.mult, OP.mult)
        sA = et("sAj1")
        nc.vector.scalar_tensor_tensor(sA[:], A[:], 1.0, A[:], OP.mult, OP.mult)
        sB2 = et("sBj1")
        nc.vector.scalar_tensor_tensor(sB2[:], Bm[:], 2.0, Bm[:], OP.mult, OP.mult)
        sD = et("sDj1")
        nc.vector.scalar_tensor_tensor(sD[:], D[:], 1.0, D[:], OP.mult, OP.mult)
        p2 = et("p2j1")
        nc.vector.tensor_add(p2[:], sA[:], sB2[:])
        nc.vector.tensor_add(p2[:], p2[:], sD[:])
        v2 = et("v2j1")
        nc.vector.tensor_sub(v2[:], q[:], p2[:])
        av = et("avj1")
        nc.scalar.activation(av[:], v2[:], AF.Abs)
        lg = et("lgj1")
        nc.scalar.activation(lg[:], av[:], AF.Ln, bias=b_eps, scale=0.5)
        nc.scalar.activation(
            outv[:, :, ch0 + 2], lg[:], AF.Exp, bias=b_ln10, scale=0.5
        )
        # ---- j = 2 : A=4 B=5 C=6 D=7 E=8 F=9
        A, Bm, Cm, D, E, F = (SM[(s, i)] for i in range(4, 10))
        sA, sB, sC, sD, sE, sF = (et(f"s{i}j2") for i in range(6))
        for dst, src in ((sA, A), (sB, Bm), (sC, Cm), (sD, D), (sE, E), (sF, F)):
            nc.vector.scalar_tensor_tensor(dst[:], src[:], 1.0, src[:], OP.mult, OP.mult)
        tAD = et("tADj2")
        nc.vector.tensor_add(tAD[:], A[:], D[:])
        p1 = et("p1j2")
        nc.vector.tensor_add(p1[:], tAD[:], F[:])
        t = et("tj2")
        nc.scalar.activation(t[:], p1[:], AF.Abs)
        nc.vector.tensor_scalar(outv[:, :, ch0 + 3], t[:], EPS, 100.0, OP.add, OP.mult)
        p2 = et("p2j2")
        nc.vector.tensor_add(p2[:], sA[:], sD[:])
        nc.vector.tensor_add(p2[:], p2[:], sF[:])
        u = et("uj2")
        nc.vector.tensor_add(u[:], sB[:], sC[:])
        nc.vector.tensor_add(u[:], u[:], sE[:])
        nc.vector.scalar_tensor_tensor(p2[:], u[:], 2.0, p2[:], OP.mult, OP.add)
        q = et("qj2")
        nc.vector.scalar_tensor_tensor(q[:], p1[:], 1.0, p1[:], OP.mult, OP.mult)
        v2 = et("v2j2")
        nc.vector.tensor_sub(v2[:], q[:], p2[:])
        av = et("avj2")
        nc.scalar.activation(av[:], v2[:], AF.Abs)
        lg = et("lgj2")
        nc.scalar.activation(lg[:], av[:], AF.Ln, bias=b_eps, scale=0.5)
        nc.scalar.activation(
            outv[:, :, ch0 + 4], lg[:], AF.Exp, bias=b_ln100, scale=0.5
        )
        # p3 = cubes + 3*(B^2(A+D) + C^2(A+F) + E^2(D+F)) + 6BCE
        cA = et("cAj2")
        nc.vector.scalar_tensor_tensor(cA[:], sA[:], 1.0, A[:], OP.mult, OP.mult)
        cD = et("cDj2")
        nc.vector.scalar_tensor_tensor(cD[:], sD[:], 1.0, D[:], OP.mult, OP.mult)
        cF = et("cFj2")
        nc.vector.scalar_tensor_tensor(cF[:], sF[:], 1.0, F[:], OP.mult, OP.mult)
        w1 = et("w1j2")
        nc.vector.tensor_add(w1[:], cA[:], cD[:])
        nc.vector.tensor_add(w1[:], w1[:], cF[:])
        y1 = et("y1j2")
        nc.vector.scalar_tensor_tensor(y1[:], sB[:], 1.0, tAD[:], OP.mult, OP.mult)
        tAF = et("tAFj2")
        nc.vector.tensor_add(tAF[:], A[:], F[:])
        y2 = et("y2j2")
        nc.vector.scalar_tensor_tensor(y2[:], sC[:], 1.0, tAF[:], OP.mult, OP.mult)
        tDF = et("tDFj2")
        nc.vector.tensor_add(tDF[:], D[:], F[:])
        y3 = et("y3j2")
        nc.vector.scalar_tensor_tensor(y3[:], sE[:], 1.0, tDF[:], OP.mult, OP.mult)
        nc.vector.tensor_add(y1[:], y1[:], y2[:])
        nc.vector.tensor_add(y1[:], y1[:], y3[:])
        z = et("zj2")
        nc.vector.scalar_tensor_tensor(z[:], Bm[:], 6.0, Cm[:], OP.mult, OP.mult)
        nc.vector.scalar_tensor_tensor(z[:], z[:], 1.0, E[:], OP.mult, OP.mult)
        nc.vector.scalar_tensor_tensor(y1[:], y1[:], 3.0, z[:], OP.mult, OP.add)
        p3 = et("p3j2")
        nc.vector.tensor_add(p3[:], w1[:], y1[:])
        # e3*3 = v2/2*p1 - p1*p2 + p3
        a3 = et("a3j2")
        nc.vector.scalar_tensor_tensor(a3[:], v2[:], 0.5, p1[:], OP.mult, OP.mult)
        b3 = et("b3j2")
        nc.vector.scalar_tensor_tensor(b3[:], p1[:], 1.0, p2[:], OP.mult, OP.mult)
        nc.vector.tensor_sub(a3[:], a3[:], b3[:])
        nc.vector.tensor_add(a3[:], a3[:], p3[:])
        nc.scalar.activation(av[:], a3[:], AF.Abs)
        nc.scalar.activation(lg[:], av[:], AF.Ln, bias=b_eps, scale=1.0 / 3.0)
        nc.scalar.activation(
            outv[:, :, ch0 + 5], lg[:], AF.Exp, bias=b_ln100, scale=1.0 / 3.0
        )

    nc.sync.dma_start(out_d[:], OUTT[:])
    for pool in (epool, opool, impool, spool, cpool):
        pool.release()


def _get_module():
    key = CONV_MODE
    if key not in _CACHE:
        _CACHE[key] = _build_module()
    return _CACHE[key]


# ---------------------------------------------------------------- entry point
def kernel(inputs, kernels0, kernels1, dg_int):
    from concourse.bass_utils import run_bass_kernel_spmd

    x = np.asarray(inputs, dtype=np.float32)
    ybands, xband, intx, inty = _build_host_tensors(kernels0, kernels1, dg_int)

    nc = _get_module()
    in_maps = []
    for core in range(8):
        b, half = core // 2, core % 2
        g0 = 0 if half == 0 else 87
        xc = np.ascontiguousarray(
            x[b, g0 : g0 + HL].transpose(0, 2, 1).reshape(HL, C * W)
        )
        in_maps.append(
            {
                "x": xc,
                "yband": ybands[half],
                "xband": xband,
                "intx": intx,
                "inty": inty,
            }
        )
    res = run_bass_kernel_spmd(nc, in_maps, core_ids=list(range(8)), **RUN_KWARGS)
    global LAST
    LAST = res
    out = np.empty((B, NWO, NWO, 12), dtype=np.float32)
    for core in range(8):
        b, half = core // 2, core % 2
        H0 = half * NOUT
        out[b, H0 : H0 + NOUT] = res.results[core]["out"].reshape(NOUT, NWO, 12)
    return out
